# revision 6
# baseline (speedup 1.0000x reference)
"""Trainium2 Bass kernel for a 2-bit-quantized (DoReFa) ResNet BasicBlock.

Full (unsharded) numpy inputs -> full numpy output.

Design notes (v2):
  - batch (64) sharded 8 images/core across 8 NeuronCores (data parallel,
    weights/BN replicated; folds done on host, O(params)).
  - 2-bit quantization makes every conv input an exact small integer.
    Activations are stored as A = 3*qa + 8 in {8,9,10,11} (fp8e4-exact).
    The +8 bias puts the value in fp8e4's [8,16) binade, whose grid
    spacing is exactly 1.0 -- so the f32->fp8 round-to-nearest-even cast
    IS the DoReFa round(3*clip(y,0,1)), bit-matching jnp.round. The
    constant 8*sum(w) per output channel folds into the BN bias.
  - each 3x3 conv = 9 shifted fp8 DoubleRow matmuls (contraction 256 via
    2 interleaved 128-chunks) accumulated in PSUM over flat plane runs.
    Planes use row stride 29: row r's right pad doubles as row r+1's
    left pad, so a 14-row half is a 405-element run (13 garbage cols).
    Measured DR throughput is 1 column/cycle at 2.4 GHz (the real fp8
    peak, 157 TF/s); LDWEIGHTS is fully hidden behind the previous
    matmul, so the PE stream is moving-column-bound.
  - conv1 epilogue: ACT computes r = Relu(P*inv1/3 + b1f) = 3*relu(bn1)
    (per-channel scale/bias native to ACT), then one DVE tensor_scalar
    (r min 3) add 8 -> fp8 writes A2 straight into the padded plane.
  - conv2 epilogue: ACT affine (P*inv2/9 + b2f), DVE residual add,
    Pool relu per half-image, store DMA triggered from the relu engine.
  - all plane pads are the constant 8: one DMA fill per plane buffer
    from a constant DRAM tensor; interiors are fully overwritten.
  - startup: image 0's x chunks are split across 4 DMA queues so the
    first matmul can start ~8us earlier; weights ride the gpsimd queue.
"""

import os
import sys
import numpy as np


def _install_ntff_hook_shim():
    """Provide antenv.axon_hooks if the image lacks it, so
    run_bass_kernel_spmd(trace=True) can capture NTFF profiles through
    libaxon_pjrt.so. No-op if the real module exists or the .so is absent."""
    try:
        import antenv.axon_hooks  # noqa: F401
        return
    except ImportError:
        pass
    import contextlib
    import ctypes
    import types

    so_path = "/opt/axon/libaxon_pjrt.so"
    _hook = None
    if os.path.exists(so_path):
        try:
            lib = ctypes.CDLL(so_path)
        except OSError:
            lib = None
        if lib is not None and hasattr(lib, "axon_start_nrt_profile"):
            lib.axon_start_nrt_profile.argtypes = [
                ctypes.POINTER(ctypes.c_int64), ctypes.c_size_t]
            lib.axon_start_nrt_profile.restype = ctypes.c_int64
            lib.axon_stop_nrt_profile.argtypes = [ctypes.c_char_p]
            lib.axon_stop_nrt_profile.restype = ctypes.c_int64

            @contextlib.contextmanager
            def _hook(output_dir, device_ids):
                import jax
                jax.devices()
                if device_ids:
                    ids = (ctypes.c_int64 * len(device_ids))(*device_ids)
                    rc = lib.axon_start_nrt_profile(ids, len(device_ids))
                else:
                    rc = lib.axon_start_nrt_profile(None, 0)
                if rc != 0:
                    raise RuntimeError(f"axon_start_nrt_profile rc={rc}")
                try:
                    yield
                finally:
                    n = lib.axon_stop_nrt_profile(str(output_dir).encode())
                    print(f"profile: {n} file(s) written to {output_dir}",
                          file=sys.stderr)

    mod = types.ModuleType("antenv.axon_hooks")
    mod.get_axon_ntff_profile_hook = lambda: _hook
    mod.set_axon_ntff_profile_hook = lambda h: None
    sys.modules["antenv.axon_hooks"] = mod


NCORES = 8
NPER = 8          # images per core
C = 256
NCH = 2           # channel chunks of 128
H = W = 28
HALF = 14         # rows per psum half
PW = 29           # plane row stride (right pad of row r == left pad of r+1)
P0 = 30           # flat offset of data pixel (0,0): one pad row + one col
QSTR = 960        # allocated plane stride (16B aligned, >= 30*29+1)
RUN = (HALF - 1) * PW + W   # 405-element flat moving run per matmul
PSH = 512         # psum f32 stride between the two halves (one bank)
BN_EPS = 1e-5


def _quant_weight3(w):
    """Replicate reference _quant_weight in f32, scaled by 3 -> {-3,-1,1,3}."""
    w = np.asarray(w, np.float32)
    t = np.tanh(w)
    m = np.max(np.abs(t))
    t2 = t / (np.float32(2.0) * m) + np.float32(0.5)
    k = np.round(t2 * np.float32(3.0))          # round-half-even == jnp.round
    return (2.0 * k - 3.0).astype(np.float32)


def _fold_bn(g, b, m, v):
    inv = np.asarray(g, np.float64) / np.sqrt(np.asarray(v, np.float64) + BN_EPS)
    beta = np.asarray(b, np.float64) - np.asarray(m, np.float64) * inv
    return inv, beta


def _w_tiles(qw3, dt):
    # [O, I, 3, 3] -> [p=128, ci=2, k=9, O=256]: lhsT slices are [128, 2, 128]
    # interleaved chunks for fp8 DoubleRow.
    return np.ascontiguousarray(
        np.transpose(qw3.reshape(C, NCH, 128, 9), (2, 1, 3, 0))
    ).astype(dt)


def _col(a):
    # [C] f64 -> [128, NCH] f32 (partition-major per chunk)
    return np.ascontiguousarray(
        np.asarray(a, np.float64).reshape(NCH, 128).T).astype(np.float32)


def _host_arrays(w1, g1, b1, m1, v1, w2, g2, b2, m2, v2):
    from concourse import mybir
    qw3_1 = _quant_weight3(w1)
    qw3_2 = _quant_weight3(w2)
    inv1, beta1 = _fold_bn(g1, b1, m1, v1)
    inv2, beta2 = _fold_bn(g2, b2, m2, v2)

    f8 = mybir.dt.np(mybir.dt.float8e4)
    w1t = _w_tiles(qw3_1, f8)
    w2t = _w_tiles(qw3_2, f8)

    # Activations A = 3*qa + 8 (padding 8), so for conv int accumulation
    # P = sum w3*A:  P_true = (P - 8*kf)/9 with kf = per-out-channel sum of
    # the scaled weights w3.
    k1f = qw3_1.reshape(C, -1).sum(axis=1).astype(np.float64)
    k2f = qw3_2.reshape(C, -1).sum(axis=1).astype(np.float64)
    # conv1 epilogue: r = Relu(P*s1 + b1f) = 3*relu(bn1(conv1)).
    s1 = _col(inv1 / 3.0)
    b1f = _col(3.0 * beta1 - (8.0 / 3.0) * k1f * inv1)
    # conv2 epilogue: u = P*s2 + b2f = bn2(conv2).
    s2 = _col(inv2 / 9.0)
    b2f = _col(beta2 - (8.0 / 9.0) * k2f * inv2)
    z8 = np.full((128, NCH, QSTR), 8.0, f8)
    return {"w1t": w1t, "w2t": w2t, "s1": s1, "b1f": b1f,
            "s2": s2, "b2f": b2f, "z8": z8}


def _build_program(nper=NPER, stage=3):
    from concourse import bacc, tile, mybir
    dt = mybir.dt
    f8 = dt.float8e4
    AF = mybir.ActivationFunctionType

    nc = bacc.Bacc("TRN2", target_bir_lowering=False, debug=False,
                   num_devices=NCORES)
    NP_ = nper

    x_d = nc.dram_tensor("x", [NP_, C, H, W], dt.float32, kind="ExternalInput")
    w1_d = nc.dram_tensor("w1t", [128, NCH, 9, C], f8, kind="ExternalInput")
    w2_d = nc.dram_tensor("w2t", [128, NCH, 9, C], f8, kind="ExternalInput")
    s1_d = nc.dram_tensor("s1", [128, NCH], dt.float32, kind="ExternalInput")
    b1_d = nc.dram_tensor("b1f", [128, NCH], dt.float32, kind="ExternalInput")
    s2_d = nc.dram_tensor("s2", [128, NCH], dt.float32, kind="ExternalInput")
    b2_d = nc.dram_tensor("b2f", [128, NCH], dt.float32, kind="ExternalInput")
    z8_d = nc.dram_tensor("z8", [128, NCH, QSTR], f8, kind="ExternalInput")
    y_d = nc.dram_tensor("y", [NP_, C, H, W], dt.float32, kind="ExternalOutput")

    with tile.TileContext(nc) as tc:
        with (
            tc.tile_pool(name="wpool", bufs=1) as wpool,
            tc.tile_pool(name="xpool", bufs=2 * NP_) as xpool,
            tc.tile_pool(name="qpool", bufs=2 * NP_) as qpool,
            tc.tile_pool(name="upool", bufs=6) as upool,
            tc.tile_pool(name="opool", bufs=6) as opool,
            tc.tile_pool(name="pspool", bufs=4, space="PSUM") as pspool,
        ):
            w1_sb = wpool.tile([128, NCH, 9, C], f8, name="w1sb")
            w2_sb = wpool.tile([128, NCH, 9, C], f8, name="w2sb")
            s1_sb = wpool.tile([128, NCH], dt.float32, name="s1sb")
            b1_sb = wpool.tile([128, NCH], dt.float32, name="b1sb")
            s2_sb = wpool.tile([128, NCH], dt.float32, name="s2sb")
            b2_sb = wpool.tile([128, NCH], dt.float32, name="b2sb")

            # 8-padded activation planes A = 3*qa + 8, one per image/conv
            qa1 = [qpool.tile([128, NCH, QSTR], f8, name=f"qa1_{n}",
                              tag="qa1") for n in range(NP_)]
            qa2 = [qpool.tile([128, NCH, QSTR], f8, name=f"qa2_{n}",
                              tag="qa2") for n in range(NP_)]
            x_sb = [[None] * NCH for _ in range(NP_)]

            def interior(qa_t, j):
                # [128, 28, 28] strided (PW, 1) view of chunk j's data area
                v = qa_t[:, j, P0:P0 + H * PW]
                return v.rearrange("p (r c) -> p r c", c=PW)[:, :, 0:W]

            # ---- startup DMA schedule ------------------------------------
            # image 0's x: both chunks split in partition halves over the
            # sync/gpsimd/scalar/vector queues for minimal latency.
            for n in range(NP_):
                for j in range(NCH):
                    x_sb[n][j] = xpool.tile([128, H, W], dt.float32,
                                            name=f"x_{n}_{j}", tag="x")
            x00, x01 = x_sb[0]
            nc.sync.dma_start(x00[0:64], x_d[0, 0:64, :, :])
            nc.gpsimd.dma_start(x00[64:128], x_d[0, 64:128, :, :])
            nc.scalar.dma_start(x01[0:64], x_d[0, 128:192, :, :])
            nc.sync.dma_start(x01[64:128], x_d[0, 192:256, :, :])
            # weights + folds + image-0 plane fill next on their queues
            nc.gpsimd.dma_start(w1_sb[:], w1_d[:])
            nc.sync.dma_start(qa1[0][:], z8_d[:])
            nc.gpsimd.dma_start(w2_sb[:], w2_d[:])
            for t_sb, t_d in ((s1_sb, s1_d), (b1_sb, b1_d),
                              (s2_sb, s2_d), (b2_sb, b2_d)):
                nc.sync.dma_start(t_sb[:], t_d[:])
            # remaining x loads and plane fills, alternating queues
            for n in range(1, NP_):
                nc.sync.dma_start(x_sb[n][0][:], x_d[n, 0:128, :, :])
                nc.gpsimd.dma_start(x_sb[n][1][:], x_d[n, 128:256, :, :])
                nc.sync.dma_start(qa2[n - 1][:], z8_d[:])
                nc.gpsimd.dma_start(qa1[n][:], z8_d[:])
            nc.sync.dma_start(qa2[NP_ - 1][:], z8_d[:])

            # ---- x quantization: A1 = (max(3x,0) min 3) + 8 --------------
            def xq_image(n):
                for j in range(NCH):
                    u = upool.tile([128, H, W], dt.float32, name="xu",
                                   tag="xu")
                    nc.scalar.activation(u[:], x_sb[n][j][:], AF.Relu,
                                         scale=3.0)
                    nc.vector.tensor_scalar(
                        interior(qa1[n], j), u[:], 3.0, 8.0,
                        mybir.AluOpType.min, mybir.AluOpType.add)

            # ---- conv: 9 shifted DR matmuls per (co, half) ---------------
            def conv_mms(ps, w_sb, qa_n, co):
                for h in range(2):
                    for k in range(9):
                        dy, dx = divmod(k, 3)
                        off = PW * (HALF * h + dy) + dx
                        nc.tensor.matmul(
                            ps[:, h * PSH:h * PSH + RUN],
                            w_sb[:, 0:NCH, k, co * 128:(co + 1) * 128],
                            qa_n[:, 0:NCH, off:off + RUN],
                            start=(k == 0), stop=(k == 8),
                            perf_mode=mybir.MatmulPerfMode.DoubleRow,
                        )

            def psv(ps):
                # [128, 2, 14, 28] valid-pixel view of a 2-bank psum tile
                return ps[:].rearrange("p (h f) -> p h f", h=2) \
                    [:, :, 0:HALF * PW] \
                    .rearrange("p h (r c) -> p h r c", c=PW)[:, :, :, 0:W]

            # conv1 -> bn1 -> relu -> requant into qa2 planes
            def conv1_image(n):
                for co in range(NCH):
                    ps = pspool.tile([128, 2 * PSH], dt.float32, name="ps1",
                                     tag="ps")
                    conv_mms(ps, w1_sb, qa1[n], co)
                    u = upool.tile([128, 2, HALF, W], dt.float32, name="c1u",
                                   tag="c1u")
                    nc.scalar.activation(u[:], psv(ps), AF.Relu,
                                         bias=b1_sb[:, co:co + 1],
                                         scale=s1_sb[:, co:co + 1])
                    out = interior(qa2[n], co) \
                        .rearrange("p (h r) c -> p h r c", h=2)
                    nc.vector.tensor_scalar(
                        out, u[:], 3.0, 8.0,
                        mybir.AluOpType.min, mybir.AluOpType.add)

            # conv2 -> bn2 -> +residual -> relu -> store
            def conv2_image(n):
                for co in range(NCH):
                    ps = pspool.tile([128, 2 * PSH], dt.float32, name="ps2",
                                     tag="ps")
                    conv_mms(ps, w2_sb, qa2[n], co)
                    u = upool.tile([128, 2, HALF, W], dt.float32, name="c2u",
                                   tag="c2u")
                    nc.scalar.activation(u[:], psv(ps), AF.Identity,
                                         bias=b2_sb[:, co:co + 1],
                                         scale=s2_sb[:, co:co + 1])
                    v = upool.tile([128, 2, HALF, W], dt.float32, name="c2v",
                                   tag="c2v")
                    xv = x_sb[n][co][:].rearrange("p (h r) c -> p h r c", h=2)
                    nc.vector.tensor_add(v[:], u[:], xv)
                    for h in range(2):
                        o = opool.tile([128, HALF, W], dt.float32, name="o",
                                       tag="o")
                        nc.gpsimd.tensor_scalar_max(o[:], v[:, h], 0.0)
                        dma = nc.gpsimd if (co + h) % 2 == 0 else nc.sync
                        dma.dma_start(
                            y_d[n, co * 128:(co + 1) * 128,
                                h * HALF:(h + 1) * HALF, :], o[:])

            def dump_qa(qa):
                # debug: copy plane interiors out as f32 (values 3*qa+8)
                for n in range(NP_):
                    for j in range(NCH):
                        o = opool.tile([128, H, W], dt.float32, name="od",
                                       tag="od")
                        nc.vector.tensor_copy(o[:], interior(qa[n], j))
                        nc.sync.dma_start(
                            y_d[n, j * 128:(j + 1) * 128, :, :], o[:])

            # ---- software-pipelined emission -----------------------------
            for n in range(NP_):
                xq_image(n)
                if stage >= 2:
                    conv1_image(n)
                if stage >= 3 and n >= 1:
                    conv2_image(n - 1)
            if stage == 1:
                dump_qa(qa1)
            if stage == 2:
                dump_qa(qa2)
            if stage >= 3:
                conv2_image(NP_ - 1)

    nc.compile()
    return nc


_CACHED = None


def _get_program():
    global _CACHED
    if _CACHED is None:
        _CACHED = _build_program(
            stage=int(os.environ.get("KERNEL_STAGE", "3")))
    return _CACHED


def kernel(x, w1, g1, b1, m1, v1, w2, g2, b2, m2, v2):
    _install_ntff_hook_shim()
    from concourse.bass_utils import run_bass_kernel_spmd

    x = np.asarray(x, np.float32)
    host = _host_arrays(w1, g1, b1, m1, v1, w2, g2, b2, m2, v2)

    xs = x.reshape(NCORES, NPER, C, H, W)
    in_maps = [{"x": np.ascontiguousarray(xs[c]), **host}
               for c in range(NCORES)]

    nc = _get_program()
    res = run_bass_kernel_spmd(
        nc, in_maps, core_ids=list(range(NCORES)),
        trace=bool(int(os.environ.get("KERNEL_TRACE", "0"))),
    )
    kernel.last_results = res
    y = np.concatenate([res.results[c]["y"][None] for c in range(NCORES)], 0)
    return np.ascontiguousarray(y.reshape(64, C, H, W).astype(np.float32))


# revision 7
# speedup vs baseline: 2.2032x; 2.2032x over previous
"""Trainium2 Bass kernel for a 2-bit-quantized (DoReFa) ResNet BasicBlock.

Full (unsharded) numpy inputs -> full numpy output.

Design notes (v2):
  - batch (64) sharded 8 images/core across 8 NeuronCores (data parallel,
    weights/BN replicated; folds done on host, O(params)).
  - 2-bit quantization makes every conv input an exact small integer.
    Activations are stored as A = 3*qa + 8 in {8,9,10,11} (fp8e4-exact).
    The +8 bias puts the value in fp8e4's [8,16) binade, whose grid
    spacing is exactly 1.0 -- so the f32->fp8 round-to-nearest-even cast
    IS the DoReFa round(3*clip(y,0,1)), bit-matching jnp.round. The
    constant 8*sum(w) per output channel folds into the BN bias.
  - each 3x3 conv = 9 shifted fp8 DoubleRow matmuls (contraction 256 via
    2 interleaved 128-chunks) accumulated in PSUM over flat plane runs.
    Planes use row stride 29: row r's right pad doubles as row r+1's
    left pad, so a 14-row half is a 405-element run (13 garbage cols).
    Measured DR throughput is 1 column/cycle at 2.4 GHz (the real fp8
    peak, 157 TF/s); LDWEIGHTS is fully hidden behind the previous
    matmul, so the PE stream is moving-column-bound.
  - conv1 epilogue: ACT computes r = Relu(P*inv1/3 + b1f) = 3*relu(bn1)
    (per-channel scale/bias native to ACT), then one DVE tensor_scalar
    (r min 3) add 8 -> fp8 writes A2 straight into the padded plane.
  - conv2 epilogue: ACT affine (P*inv2/9 + b2f), DVE residual add,
    Pool relu per half-image, store DMA triggered from the relu engine.
  - all plane pads are the constant 8: one DMA fill per plane buffer
    from a constant DRAM tensor; interiors are fully overwritten.
  - startup: image 0's x chunks are split across 4 DMA queues so the
    first matmul can start ~8us earlier; weights ride the gpsimd queue.
"""

import os
import sys
import numpy as np


def _install_ntff_hook_shim():
    """Provide antenv.axon_hooks if the image lacks it, so
    run_bass_kernel_spmd(trace=True) can capture NTFF profiles through
    libaxon_pjrt.so. No-op if the real module exists or the .so is absent."""
    try:
        import antenv.axon_hooks  # noqa: F401
        return
    except ImportError:
        pass
    import contextlib
    import ctypes
    import types

    so_path = "/opt/axon/libaxon_pjrt.so"
    _hook = None
    if os.path.exists(so_path):
        try:
            lib = ctypes.CDLL(so_path)
        except OSError:
            lib = None
        if lib is not None and hasattr(lib, "axon_start_nrt_profile"):
            lib.axon_start_nrt_profile.argtypes = [
                ctypes.POINTER(ctypes.c_int64), ctypes.c_size_t]
            lib.axon_start_nrt_profile.restype = ctypes.c_int64
            lib.axon_stop_nrt_profile.argtypes = [ctypes.c_char_p]
            lib.axon_stop_nrt_profile.restype = ctypes.c_int64

            @contextlib.contextmanager
            def _hook(output_dir, device_ids):
                import jax
                jax.devices()
                if device_ids:
                    ids = (ctypes.c_int64 * len(device_ids))(*device_ids)
                    rc = lib.axon_start_nrt_profile(ids, len(device_ids))
                else:
                    rc = lib.axon_start_nrt_profile(None, 0)
                if rc != 0:
                    raise RuntimeError(f"axon_start_nrt_profile rc={rc}")
                try:
                    yield
                finally:
                    n = lib.axon_stop_nrt_profile(str(output_dir).encode())
                    print(f"profile: {n} file(s) written to {output_dir}",
                          file=sys.stderr)

    mod = types.ModuleType("antenv.axon_hooks")
    mod.get_axon_ntff_profile_hook = lambda: _hook
    mod.set_axon_ntff_profile_hook = lambda h: None
    sys.modules["antenv.axon_hooks"] = mod


NCORES = 8
NPER = 8          # images per core
C = 256
NCH = 2           # channel chunks of 128
H = W = 28
HALF = 14         # rows per psum half
PW = 29           # plane row stride (right pad of row r == left pad of r+1)
P0 = 30           # flat offset of data pixel (0,0): one pad row + one col
QSTR = 960        # allocated plane stride (16B aligned, >= 30*29+1)
RUN = (HALF - 1) * PW + W   # 405-element flat moving run per matmul
PSH = 512         # psum f32 stride between the two halves (one bank)
BN_EPS = 1e-5


def _quant_weight3(w):
    """Replicate reference _quant_weight in f32, scaled by 3 -> {-3,-1,1,3}."""
    w = np.asarray(w, np.float32)
    t = np.tanh(w)
    m = np.max(np.abs(t))
    t2 = t / (np.float32(2.0) * m) + np.float32(0.5)
    k = np.round(t2 * np.float32(3.0))          # round-half-even == jnp.round
    return (2.0 * k - 3.0).astype(np.float32)


def _fold_bn(g, b, m, v):
    inv = np.asarray(g, np.float64) / np.sqrt(np.asarray(v, np.float64) + BN_EPS)
    beta = np.asarray(b, np.float64) - np.asarray(m, np.float64) * inv
    return inv, beta


def _w_tiles(qw3, dt):
    # [O, I, 3, 3] -> [p=128, ci=2, k=9, O=256]: lhsT slices are [128, 2, 128]
    # interleaved chunks for fp8 DoubleRow.
    return np.ascontiguousarray(
        np.transpose(qw3.reshape(C, NCH, 128, 9), (2, 1, 3, 0))
    ).astype(dt)


def _col(a):
    # [C] f64 -> [128, NCH] f32 (partition-major per chunk)
    return np.ascontiguousarray(
        np.asarray(a, np.float64).reshape(NCH, 128).T).astype(np.float32)


def _host_arrays(w1, g1, b1, m1, v1, w2, g2, b2, m2, v2):
    from concourse import mybir
    qw3_1 = _quant_weight3(w1)
    qw3_2 = _quant_weight3(w2)
    inv1, beta1 = _fold_bn(g1, b1, m1, v1)
    inv2, beta2 = _fold_bn(g2, b2, m2, v2)

    f8 = mybir.dt.np(mybir.dt.float8e4)
    w1t = _w_tiles(qw3_1, f8)
    w2t = _w_tiles(qw3_2, f8)

    # Activations A = 3*qa + 8 (padding 8), so for conv int accumulation
    # P = sum w3*A:  P_true = (P - 8*kf)/9 with kf = per-out-channel sum of
    # the scaled weights w3.
    k1f = qw3_1.reshape(C, -1).sum(axis=1).astype(np.float64)
    k2f = qw3_2.reshape(C, -1).sum(axis=1).astype(np.float64)
    # conv1 epilogue: r = Relu(P*s1 + b1f) = 3*relu(bn1(conv1)).
    s1 = _col(inv1 / 3.0)
    b1f = _col(3.0 * beta1 - (8.0 / 3.0) * k1f * inv1)
    # conv2 epilogue: u = P*s2 + b2f = bn2(conv2).
    s2 = _col(inv2 / 9.0)
    b2f = _col(beta2 - (8.0 / 9.0) * k2f * inv2)
    z8 = np.full((128, NCH, QSTR), 8.0, f8)
    return {"w1t": w1t, "w2t": w2t, "s1": s1, "b1f": b1f,
            "s2": s2, "b2f": b2f, "z8": z8}


def _build_program(nper=NPER, stage=3):
    from concourse import bacc, tile, mybir
    dt = mybir.dt
    f8 = dt.float8e4
    AF = mybir.ActivationFunctionType

    nc = bacc.Bacc("TRN2", target_bir_lowering=False, debug=False,
                   num_devices=NCORES)
    NP_ = nper

    x_d = nc.dram_tensor("x", [NP_, C, H, W], dt.float32, kind="ExternalInput")
    w1_d = nc.dram_tensor("w1t", [128, NCH, 9, C], f8, kind="ExternalInput")
    w2_d = nc.dram_tensor("w2t", [128, NCH, 9, C], f8, kind="ExternalInput")
    s1_d = nc.dram_tensor("s1", [128, NCH], dt.float32, kind="ExternalInput")
    b1_d = nc.dram_tensor("b1f", [128, NCH], dt.float32, kind="ExternalInput")
    s2_d = nc.dram_tensor("s2", [128, NCH], dt.float32, kind="ExternalInput")
    b2_d = nc.dram_tensor("b2f", [128, NCH], dt.float32, kind="ExternalInput")
    z8_d = nc.dram_tensor("z8", [128, NCH, QSTR], f8, kind="ExternalInput")
    y_d = nc.dram_tensor("y", [NP_, C, H, W], dt.float32, kind="ExternalOutput")

    with tile.TileContext(nc) as tc:
        with (
            tc.tile_pool(name="wpool", bufs=1) as wpool,
            tc.tile_pool(name="xpool", bufs=2 * NP_) as xpool,
            tc.tile_pool(name="qpool", bufs=2 * NP_) as qpool,
            tc.tile_pool(name="upool", bufs=6) as upool,
            tc.tile_pool(name="opool", bufs=6) as opool,
            tc.tile_pool(name="pspool", bufs=4, space="PSUM") as pspool,
        ):
            w1_sb = wpool.tile([128, NCH, 9, C], f8, name="w1sb")
            w2_sb = wpool.tile([128, NCH, 9, C], f8, name="w2sb")
            s1_sb = wpool.tile([128, NCH], dt.float32, name="s1sb")
            b1_sb = wpool.tile([128, NCH], dt.float32, name="b1sb")
            s2_sb = wpool.tile([128, NCH], dt.float32, name="s2sb")
            b2_sb = wpool.tile([128, NCH], dt.float32, name="b2sb")

            # 8-padded activation planes A = 3*qa + 8, one per image/conv
            qa1 = [qpool.tile([128, NCH, QSTR], f8, name=f"qa1_{n}",
                              tag="qa1") for n in range(NP_)]
            qa2 = [qpool.tile([128, NCH, QSTR], f8, name=f"qa2_{n}",
                              tag="qa2") for n in range(NP_)]
            x_sb = [[None] * NCH for _ in range(NP_)]

            def interior(qa_t, j):
                # [128, 28, 28] strided (PW, 1) view of chunk j's data area
                v = qa_t[:, j, P0:P0 + H * PW]
                return v.rearrange("p (r c) -> p r c", c=PW)[:, :, 0:W]

            # ---- startup DMA schedule ------------------------------------
            # image 0's x: both chunks split in partition halves over the
            # sync/gpsimd/scalar/vector queues for minimal latency.
            for n in range(NP_):
                for j in range(NCH):
                    x_sb[n][j] = xpool.tile([128, H, W], dt.float32,
                                            name=f"x_{n}_{j}", tag="x")
            x00, x01 = x_sb[0]
            nc.sync.dma_start(x00[0:64], x_d[0, 0:64, :, :])
            nc.gpsimd.dma_start(x00[64:128], x_d[0, 64:128, :, :])
            nc.scalar.dma_start(x01[0:64], x_d[0, 128:192, :, :])
            nc.sync.dma_start(x01[64:128], x_d[0, 192:256, :, :])
            # weights + folds + image-0 plane fill next on their queues
            nc.gpsimd.dma_start(w1_sb[:], w1_d[:])
            nc.sync.dma_start(qa1[0][:], z8_d[:])
            nc.gpsimd.dma_start(w2_sb[:], w2_d[:])
            for t_sb, t_d in ((s1_sb, s1_d), (b1_sb, b1_d),
                              (s2_sb, s2_d), (b2_sb, b2_d)):
                nc.sync.dma_start(t_sb[:], t_d[:])
            # remaining x loads and plane fills, alternating queues
            for n in range(1, NP_):
                nc.sync.dma_start(x_sb[n][0][:], x_d[n, 0:128, :, :])
                nc.gpsimd.dma_start(x_sb[n][1][:], x_d[n, 128:256, :, :])
                nc.sync.dma_start(qa2[n - 1][:], z8_d[:])
                nc.gpsimd.dma_start(qa1[n][:], z8_d[:])
            nc.sync.dma_start(qa2[NP_ - 1][:], z8_d[:])

            # ---- x quantization: A1 = (max(3x,0) min 3) + 8 --------------
            def xq_image(n):
                for j in range(NCH):
                    u = upool.tile([128, H, W], dt.float32, name="xu",
                                   tag="xu")
                    nc.scalar.activation(u[:], x_sb[n][j][:], AF.Relu,
                                         scale=3.0)
                    nc.vector.tensor_scalar(
                        interior(qa1[n], j), u[:], 3.0, 8.0,
                        mybir.AluOpType.min, mybir.AluOpType.add)

            # ---- conv: 9 shifted DR matmuls per (co, half) ---------------
            def conv_mms(ps, w_sb, qa_n, co):
                for h in range(2):
                    for k in range(9):
                        dy, dx = divmod(k, 3)
                        off = PW * (HALF * h + dy) + dx
                        nc.tensor.matmul(
                            ps[:, h * PSH:h * PSH + RUN],
                            w_sb[:, 0:NCH, k, co * 128:(co + 1) * 128],
                            qa_n[:, 0:NCH, off:off + RUN],
                            start=(k == 0), stop=(k == 8),
                            perf_mode=mybir.MatmulPerfMode.DoubleRow,
                        )

            def psv(ps):
                # [128, 2, 14, 28] valid-pixel view of a 2-bank psum tile
                return ps[:].rearrange("p (h f) -> p h f", h=2) \
                    [:, :, 0:HALF * PW] \
                    .rearrange("p h (r c) -> p h r c", c=PW)[:, :, :, 0:W]

            # conv1 -> bn1 -> relu -> requant into qa2 planes
            def conv1_image(n):
                for co in range(NCH):
                    ps = pspool.tile([128, 2 * PSH], dt.float32, name="ps1",
                                     tag="ps")
                    conv_mms(ps, w1_sb, qa1[n], co)
                    u = upool.tile([128, 2, HALF, W], dt.float32, name="c1u",
                                   tag="c1u")
                    nc.scalar.activation(u[:], psv(ps), AF.Relu,
                                         bias=b1_sb[:, co:co + 1],
                                         scale=s1_sb[:, co:co + 1])
                    out = interior(qa2[n], co) \
                        .rearrange("p (h r) c -> p h r c", h=2)
                    nc.vector.tensor_scalar(
                        out, u[:], 3.0, 8.0,
                        mybir.AluOpType.min, mybir.AluOpType.add)

            # conv2 -> bn2 -> +residual -> relu -> store
            def conv2_image(n):
                for co in range(NCH):
                    ps = pspool.tile([128, 2 * PSH], dt.float32, name="ps2",
                                     tag="ps")
                    conv_mms(ps, w2_sb, qa2[n], co)
                    u = upool.tile([128, 2, HALF, W], dt.float32, name="c2u",
                                   tag="c2u")
                    nc.scalar.activation(u[:], psv(ps), AF.Identity,
                                         bias=b2_sb[:, co:co + 1],
                                         scale=s2_sb[:, co:co + 1])
                    v = upool.tile([128, 2, HALF, W], dt.float32, name="c2v",
                                   tag="c2v")
                    xv = x_sb[n][co][:].rearrange("p (h r) c -> p h r c", h=2)
                    nc.vector.tensor_add(v[:], u[:], xv)
                    for h in range(2):
                        o = opool.tile([128, HALF, W], dt.float32, name="o",
                                       tag="o")
                        if co == 0:
                            nc.vector.tensor_scalar_max(o[:], v[:, h], 0.0)
                        else:
                            nc.scalar.activation(o[:], v[:, h], AF.Relu)
                        dma = nc.gpsimd if (co + h) % 2 == 0 else nc.sync
                        dma.dma_start(
                            y_d[n, co * 128:(co + 1) * 128,
                                h * HALF:(h + 1) * HALF, :], o[:])

            def dump_qa(qa):
                # debug: copy plane interiors out as f32 (values 3*qa+8)
                for n in range(NP_):
                    for j in range(NCH):
                        o = opool.tile([128, H, W], dt.float32, name="od",
                                       tag="od")
                        nc.vector.tensor_copy(o[:], interior(qa[n], j))
                        nc.sync.dma_start(
                            y_d[n, j * 128:(j + 1) * 128, :, :], o[:])

            # ---- software-pipelined emission -----------------------------
            for n in range(NP_):
                xq_image(n)
                if stage >= 2:
                    conv1_image(n)
                if stage >= 3 and n >= 1:
                    conv2_image(n - 1)
            if stage == 1:
                dump_qa(qa1)
            if stage == 2:
                dump_qa(qa2)
            if stage >= 3:
                conv2_image(NP_ - 1)

    nc.compile()
    return nc


_CACHED = None


def _get_program():
    global _CACHED
    if _CACHED is None:
        _CACHED = _build_program(
            stage=int(os.environ.get("KERNEL_STAGE", "3")))
    return _CACHED


def kernel(x, w1, g1, b1, m1, v1, w2, g2, b2, m2, v2):
    _install_ntff_hook_shim()
    from concourse.bass_utils import run_bass_kernel_spmd

    x = np.asarray(x, np.float32)
    host = _host_arrays(w1, g1, b1, m1, v1, w2, g2, b2, m2, v2)

    xs = x.reshape(NCORES, NPER, C, H, W)
    in_maps = [{"x": np.ascontiguousarray(xs[c]), **host}
               for c in range(NCORES)]

    nc = _get_program()
    res = run_bass_kernel_spmd(
        nc, in_maps, core_ids=list(range(NCORES)),
        trace=bool(int(os.environ.get("KERNEL_TRACE", "0"))),
    )
    kernel.last_results = res
    y = np.concatenate([res.results[c]["y"][None] for c in range(NCORES)], 0)
    return np.ascontiguousarray(y.reshape(64, C, H, W).astype(np.float32))
